# revision 11
# baseline (speedup 1.0000x reference)
"""Bidirectional Chamfer distance on 8 Trainium2 NeuronCores (v3).

Problem: B=4 batches, N=M=4096 3-D points, f32.
  dist[b,n,m] = ||s[b,n]-t[b,m]||^2
  loss = mean_b( mean_n min_m dist + mean_m min_n dist )

Sharding: core c handles batch b=c//2, source-row half hh=c%2
(2048 source rows x 4096 target cols per core).  All device math runs in
NEGATED-distance space (PE emits -dist via bf16 hi/lo augmented matmuls,
fp32-exact), so every reduction is a max.

v3 engine layout, per (h, nt) tile of [128 src rows x 2048 tgt cols]:
  PE    4 matmuls -> ps (PSUM fp32)                      ~0.85us
  drain split at column SPLIT=1692: ACT copies ps[:, :SPLIT] -> cph fp16
        (1.60us, the pipeline cadence); DVE drains the rest fused
        with its rowmax accum (frees PSUM in lockstep with ACT)
  row   DVE tensor_scalar 4x accum over cph[:, :SPLIT] (out -> junk buf)
  col   three routes, balancing DVE/SP-DMA/Pool:
        - transposed tiles (nt 0..3 per half): SP-issued xbar transpose
          of cph[:, :1792] into V[h][128, 14, r]; per-column-class
          colmins come from staged DVE tensor_scalar 4x accums over V
        - pool tiles (nt 4..15, both halves): gpsimd partition_all_reduce
          of cph[:, :1792] directly (128-row max per column), keeping the
          transpose+V-reduce tail off the critical path
        - chain blocks (k=14,15): DVE tensor_tensor fp16 chain into acc,
          finished per-half by one partition_all_reduce
Stage reduces are throttled (<=3 per tile) and deferred behind each
tile's fused drain so DVE's in-order stream never stalls PSUM rotation.
CoreSim cost model: ~60.7us per core (ACT ~52us cadence-bound; DVE ~48,
SP-DMA ~25, Pool ~41 busy).
"""

import numpy as np
import ml_dtypes

B, N, M = 4, 4096, 4096
N_CORES = 8
NSH = N // 2          # 2048 source rows per core
K = 16                # augmented contraction dim
NT = NSH // 128       # 16 stationary tiles per half
HALVES = 2

TK = 14               # k-blocks (128 cols each) routed via transpose/pool
CK = 16 - TK          # k-blocks routed via the fp16 chain
SPLIT = 1692          # per-tile drain split column (ACT below, DVE above)
SPREAD = 3            # max stage reduces injected per tile step
VRED_DELAY = 2        # tiles to defer a completed stage's reduces
COPY_BUFS = 8
CHAIN_INIT = -60000.0
# V-reduce stages per half: list of (nt_end, r_len)
STAGES = {0: [(2, 256), (4, 256)], 1: [(2, 256), (4, 256)]}
# pool-routed tiles: columns [0, TK*128) reduced by partition_all_reduce
TAIL_TILES = (tuple((0, x) for x in range(4, 16))
              + tuple((1, x) for x in range(4, 16)))

_PROGRAM = None


def _build_program():
    import concourse.mybir as mybir
    import concourse.tile as tile
    from concourse import bacc, bass_isa
    from contextlib import ExitStack

    nc = bacc.Bacc(name="chamfer3")
    f32 = mybir.dt.float32
    f16 = mybir.dt.float16
    bf16 = mybir.dt.bfloat16

    nrow_slots = HALVES * NT * 2
    n_col_slots = sum(len(v) for v in STAGES.values()) * TK

    saugT = nc.dram_tensor("saugT", [K, NSH], bf16, kind="ExternalInput")
    taugT = nc.dram_tensor("taugT", [K, M], bf16, kind="ExternalInput")
    out_row = nc.dram_tensor("out_row", [128, nrow_slots], f32, kind="ExternalOutput")
    out_col = nc.dram_tensor("out_col", [128, n_col_slots], f32, kind="ExternalOutput")
    out_chain = nc.dram_tensor("out_chain", [1, HALVES * CK * 128], f16,
                               kind="ExternalOutput")
    out_tail = nc.dram_tensor("out_tail", [1, len(TAIL_TILES) * 2048], f16,
                              kind="ExternalOutput")

    with tile.TileContext(nc) as tc, ExitStack() as ctx:
        inputs = ctx.enter_context(tc.tile_pool(name="inputs", bufs=1))
        psum_pool = ctx.enter_context(tc.tile_pool(name="psum", bufs=2, space="PSUM"))
        copy_pool = ctx.enter_context(tc.tile_pool(name="copies", bufs=COPY_BUFS))
        vpool = ctx.enter_context(tc.tile_pool(name="vpool", bufs=1))
        outp = ctx.enter_context(tc.tile_pool(name="outp", bufs=1))

        saug = inputs.tile([K, NSH], bf16)
        taug = inputs.tile([K, M], bf16)
        # first-needed slices via the fast SP HWDGE path, rest on gpsimd
        nc.sync.dma_start(out=saug[:, :128], in_=saugT[:, :128])
        nc.sync.dma_start(out=taug[:, :512], in_=taugT[:, :512])
        nc.sync.dma_start(out=taug[:, 512:2048], in_=taugT[:, 512:2048])
        nc.gpsimd.dma_start(out=saug[:, 128:], in_=saugT[:, 128:])
        for i in range(2, 4):
            nc.gpsimd.dma_start(
                out=taug[:, i * (M // 4):(i + 1) * (M // 4)],
                in_=taugT[:, i * (M // 4):(i + 1) * (M // 4)],
            )

        V = [vpool.tile([128, TK, NSH], f16, name=f"V{h}") for h in range(HALVES)]
        junk = vpool.tile([128, 2048], f16)          # dead-write sink
        # touch the scalar engine immediately so its activation-table load
        # (1.3us) overlaps the input DMAs instead of delaying the 1st drain
        nc.vector.memset(junk[:, 0:8], 0.0)
        nc.scalar.copy(out=junk[:, 0:8], in_=junk[:, 0:8])

        rowpart = outp.tile([128, nrow_slots], f32)
        colpart = outp.tile([128, n_col_slots], f32)
        acc = vpool.tile([128, HALVES, CK * 128], f16)
        accB = vpool.tile([128, HALVES, CK * 128], f16)
        nc.gpsimd.memset(acc, CHAIN_INIT)

        pending = []          # (due_step, h, base_slot, k, r0, rl)
        col_slot = 0
        stage_idx = {0: 0, 1: 0}
        r_done = {0: 0, 1: 0}
        step = 0

        def flush_pending(now, force=False):
            nonlocal pending
            budget = len(pending) if force else SPREAD
            emitted = 0
            keep = []
            for (due, h, base, k, r0, rl) in pending:
                if (force or now >= due) and emitted < budget:
                    nc.vector.tensor_scalar(
                        out=junk[:, :rl],
                        in0=V[h][:, k, r0:r0 + rl],
                        scalar1=0.0,
                        scalar2=None,
                        op0=mybir.AluOpType.add,
                        op1=mybir.AluOpType.max,
                        accum_out=colpart[:, base + k:base + k + 1],
                    )
                    emitted += 1
                else:
                    keep.append((due, h, base, k, r0, rl))
            pending = keep

        deferred = []   # postponed DVE post-work: (rowmax_ap, slot, cph, h, chain)

        def emit_deferred():
            for (dmax, dslot, dcph, dh, dchain) in deferred:
                nc.vector.tensor_scalar(
                    out=junk[:, :SPLIT], in0=dmax, scalar1=0.0,
                    scalar2=None, op0=mybir.AluOpType.add,
                    op1=mybir.AluOpType.max, accum_out=dslot,
                )
                if dchain:
                    # chain only covers transposed tiles; pool tiles'
                    # partition reduce spans the full 2048 columns
                    a_sl = acc[:, dh, :]
                    nc.vector.tensor_tensor(
                        out=a_sl, in0=dcph[:, TK * 128:], in1=a_sl,
                        op=mybir.AluOpType.max,
                    )
            deferred.clear()

        tail_sorted = sorted(TAIL_TILES)
        for h in range(HALVES):
            for nt in range(NT):
                ps = psum_pool.tile([128, 2048], f32, tag="ps")
                for q in range(4):
                    mt = h * 4 + q
                    nc.tensor.matmul(
                        ps[:, q * 512:(q + 1) * 512],
                        saug[:, nt * 128:(nt + 1) * 128],
                        taug[:, mt * 512:(mt + 1) * 512],
                        start=True,
                        stop=True,
                    )
                cph = copy_pool.tile([128, 2048], f16, tag="cph")
                t_i = h * NT + nt
                slotA = rowpart[:, 2 * t_i:2 * t_i + 1]
                slotB = rowpart[:, 2 * t_i + 1:2 * t_i + 2]
                # DVE fused drain of the tail columns first (frees PSUM in
                # lockstep with the ACT drain of the head columns)
                nc.vector.tensor_scalar(
                    out=cph[:, SPLIT:], in0=ps[:, SPLIT:], scalar1=0.0,
                    scalar2=None, op0=mybir.AluOpType.add,
                    op1=mybir.AluOpType.max, accum_out=slotB,
                )
                nc.scalar.copy(out=cph[:, :SPLIT], in_=ps[:, :SPLIT])
                emit_deferred()
                is_pool = (h, nt) in TAIL_TILES
                deferred.append((cph[:, :SPLIT], slotA, cph, h, not is_pool))

                if is_pool:
                    ti = tail_sorted.index((h, nt))
                    tred = copy_pool.tile([128, 2048], f16, tag="tred")
                    nc.gpsimd.partition_all_reduce(
                        tred, cph, 128, bass_isa.ReduceOp.max
                    )
                    nc.sync.dma_start(
                        out=out_tail[0:1, ti * 2048:(ti + 1) * 2048],
                        in_=tred[0:1, :],
                    )
                else:
                    nc.sync.dma_start_transpose(
                        out=V[h][:, :, nt * 128:(nt + 1) * 128],
                        in_=cph[:, :TK * 128],
                    )

                step += 1
                if stage_idx[h] < len(STAGES[h]):
                    nt_end, r_len = STAGES[h][stage_idx[h]]
                    if nt + 1 == nt_end:
                        for k in range(TK):
                            pending.append((step + VRED_DELAY, h, col_slot,
                                            k, r_done[h], r_len))
                        col_slot += TK
                        r_done[h] += r_len
                        stage_idx[h] += 1
                flush_pending(step)

                if nt == NT - 1:
                    # this half's chain is complete: partition-reduce it now
                    emit_deferred()
                    nc.gpsimd.partition_all_reduce(
                        accB[:, h, :], acc[:, h, :], 128, bass_isa.ReduceOp.max
                    )
                    nc.sync.dma_start(
                        out=out_chain[0:1, h * CK * 128:(h + 1) * CK * 128],
                        in_=accB[0:1, h, :],
                    )

        emit_deferred()
        flush_pending(step, force=True)
        nc.sync.dma_start(out=out_row[:, :], in_=rowpart)
        nc.sync.dma_start(out=out_col[:, :], in_=colpart)

    nc.finalize()
    return nc


def _augment(source, target):
    """Per-core augmented bf16 hi/lo operands (NEGATED-distance space)."""
    bf = ml_dtypes.bfloat16

    def split(x):
        hi = x.astype(bf)
        lo = (x - hi.astype(np.float32)).astype(bf)
        return hi, lo

    in_maps = []
    for c in range(N_CORES):
        b, hh = c // 2, c % 2
        s = np.asarray(source[b, hh * NSH:(hh + 1) * NSH], dtype=np.float32)
        t = np.asarray(target[b], dtype=np.float32)
        a = 2.0 * s
        ns = -(s * s).sum(axis=1, dtype=np.float32)
        ntg = (t * t).sum(axis=1, dtype=np.float32)
        ah, al = split(a)
        th, tl = split(t)
        nsh_, nsl = split(ns)
        nth, ntl = split(ntg)
        ones_s = np.ones(NSH, dtype=bf)
        ones_t = np.ones(M, dtype=bf)

        sa = np.empty((K, NSH), dtype=bf)
        ta = np.empty((K, M), dtype=bf)
        sa[0:3] = ah.T
        ta[0:3] = th.T
        sa[3:6] = ah.T
        ta[3:6] = tl.T
        sa[6:9] = al.T
        ta[6:9] = th.T
        sa[9:12] = al.T
        ta[9:12] = tl.T
        sa[12] = nsh_
        sa[13] = nsl
        ta[12] = ones_t
        ta[13] = ones_t
        sa[14] = -ones_s
        sa[15] = -ones_s
        ta[14] = nth
        ta[15] = ntl

        in_maps.append({"saugT": sa, "taugT": ta})
    return in_maps


# test harness hook: set _BENCH["trace"]=True to profile; results land in
# _BENCH["last"] (BassKernelResults with exec_time_ns).
_BENCH = {"trace": False, "last": None}


def _core_mins(res):
    """Decode one core's outputs -> (rowmax_neg [NSH], colmax_neg [M])."""
    # rows: two slots per tile (ACT part / DVE fused part)
    rp = res["out_row"]                       # (128, 2*32)
    tile_rowmax = np.maximum(rp[:, 0::2], rp[:, 1::2])   # (128, 32) t=h*NT+nt
    # row r of half h lives at tile h*NT + r//128, partition r%128
    rowmax = np.maximum(tile_rowmax[:, :NT], tile_rowmax[:, NT:])  # (128, NT)
    rowmax_neg = rowmax.T.reshape(-1)         # (NSH,) ordered by r

    # columns
    cp = res["out_col"].astype(np.float32)    # (128, n_col_slots)
    tl = res["out_tail"][0].astype(np.float32)
    ch = res["out_chain"][0].astype(np.float32)
    colmax = np.full((M,), -np.inf, np.float32)
    # V-stage slot bases follow build order: h=0 stages, then h=1
    slot_base = {}
    csl = 0
    for h in range(HALVES):
        for s_i in range(len(STAGES[h])):
            slot_base[(h, s_i)] = csl
            csl += TK
    for h in range(HALVES):
        part = np.full((128, TK), -np.inf, np.float32)
        for s_i in range(len(STAGES[h])):
            base = slot_base[(h, s_i)]
            part = np.maximum(part, cp[:, base:base + TK])
        # col = h*2048 + k*128 + p  ->  part[p, k]
        colmax[h * 2048:h * 2048 + TK * 128] = part.T.reshape(-1)
        # chain blocks k in [TK, 16)
        colmax[h * 2048 + TK * 128:(h + 1) * 2048] = ch[h * CK * 128:(h + 1) * CK * 128]
    # pool-routed tiles: out_tail[ti] covers cols [h*2048, h*2048+TK*128)
    for ti, (h, nt) in enumerate(sorted(TAIL_TILES)):
        seg = tl[ti * 2048:(ti + 1) * 2048]
        sl = slice(h * 2048, (h + 1) * 2048)
        colmax[sl] = np.maximum(colmax[sl], seg)
    return rowmax_neg, colmax


def kernel(source, target):
    global _PROGRAM
    from concourse.bass_utils import run_bass_kernel_spmd

    source = np.asarray(source, dtype=np.float32)
    target = np.asarray(target, dtype=np.float32)

    if _PROGRAM is None:
        _PROGRAM = _build_program()

    in_maps = _augment(source, target)
    bkr = run_bass_kernel_spmd(
        _PROGRAM, in_maps, list(range(N_CORES)), trace=_BENCH["trace"]
    )
    _BENCH["last"] = bkr
    res = bkr.results

    loss = np.float64(0.0)
    for b in range(B):
        r0_row, r0_col = _core_mins(res[2 * b])
        r1_row, r1_col = _core_mins(res[2 * b + 1])
        rowmin = -np.concatenate([r0_row, r1_row])        # (N,)
        colmin = -np.maximum(r0_col, r1_col)              # (M,)
        loss += rowmin.mean(dtype=np.float64) + colmin.mean(dtype=np.float64)
    return np.float32(loss / B)


# revision 12
# speedup vs baseline: 1.0304x; 1.0304x over previous
"""Bidirectional Chamfer distance on 8 Trainium2 NeuronCores (v3).

Problem: B=4 batches, N=M=4096 3-D points, f32.
  dist[b,n,m] = ||s[b,n]-t[b,m]||^2
  loss = mean_b( mean_n min_m dist + mean_m min_n dist )

Sharding: core c handles batch b=c//2, source-row half hh=c%2
(2048 source rows x 4096 target cols per core).  All device math runs in
NEGATED-distance space (PE emits -dist via bf16 hi/lo augmented matmuls,
fp32-exact), so every reduction is a max.

v3 engine layout, per (h, nt) tile of [128 src rows x 2048 tgt cols]:
  PE    4 matmuls -> ps (PSUM fp32)                      ~0.85us
  drain split at column SPLIT=1692: ACT copies ps[:, :SPLIT] -> cph fp16
        (1.60us, the pipeline cadence); DVE drains the rest fused
        with its rowmax accum (frees PSUM in lockstep with ACT)
  row   DVE tensor_scalar 4x accum over cph[:, :SPLIT] (out -> junk buf)
  col   three routes, balancing DVE/SP-DMA/Pool:
        - transposed tiles (nt 0..3 per half): SP-issued xbar transpose
          of cph[:, :1792] into V[h][128, 14, r]; per-column-class
          colmins come from staged DVE tensor_scalar 4x accums over V
        - pool tiles (nt 4..15, both halves): gpsimd partition_all_reduce
          of cph[:, :1792] directly (128-row max per column), keeping the
          transpose+V-reduce tail off the critical path
        - chain blocks (k=14,15): DVE tensor_tensor fp16 chain into acc,
          finished per-half by one partition_all_reduce
Stage reduces are throttled (<=3 per tile) and deferred behind each
tile's fused drain so DVE's in-order stream never stalls PSUM rotation.
CoreSim cost model: ~60.7us per core (ACT ~52us cadence-bound; DVE ~48,
SP-DMA ~25, Pool ~41 busy).
"""

import numpy as np
import ml_dtypes

B, N, M = 4, 4096, 4096
N_CORES = 8
NSH = N // 2          # 2048 source rows per core
K = 16                # augmented contraction dim
NT = NSH // 128       # 16 stationary tiles per half
HALVES = 2

TK = 14               # k-blocks (128 cols each) routed via transpose/pool
CK = 16 - TK          # k-blocks routed via the fp16 chain
SPLIT = 1692          # per-tile drain split column (ACT below, DVE above)
SPREAD = 3            # max stage reduces injected per tile step
VRED_DELAY = 2        # tiles to defer a completed stage's reduces
COPY_BUFS = 8
CHAIN_INIT = -60000.0
# V-reduce stages per half: list of (nt_end, r_len)
STAGES = {0: [(2, 256), (4, 256)], 1: [(2, 256), (4, 256)]}
# pool-routed tiles: columns [0, TK*128) reduced by partition_all_reduce
TAIL_TILES = (tuple((0, x) for x in range(4, 16))
              + tuple((1, x) for x in range(4, 16)))

_PROGRAM = None


def _build_program():
    import concourse.mybir as mybir
    import concourse.tile as tile
    from concourse import bacc, bass_isa
    from contextlib import ExitStack

    nc = bacc.Bacc(name="chamfer3")
    f32 = mybir.dt.float32
    f16 = mybir.dt.float16
    bf16 = mybir.dt.bfloat16

    nrow_slots = HALVES * NT * 2
    n_col_slots = sum(len(v) for v in STAGES.values()) * TK

    saugT = nc.dram_tensor("saugT", [K, NSH], bf16, kind="ExternalInput")
    taugT = nc.dram_tensor("taugT", [K, M], bf16, kind="ExternalInput")
    out_row = nc.dram_tensor("out_row", [128, nrow_slots], f32, kind="ExternalOutput")
    out_col = nc.dram_tensor("out_col", [128, n_col_slots], f32, kind="ExternalOutput")
    out_chain = nc.dram_tensor("out_chain", [1, HALVES * CK * 128], f16,
                               kind="ExternalOutput")
    out_tail = nc.dram_tensor("out_tail", [1, len(TAIL_TILES) * TK * 128], f16,
                              kind="ExternalOutput")

    with tile.TileContext(nc) as tc, ExitStack() as ctx:
        inputs = ctx.enter_context(tc.tile_pool(name="inputs", bufs=1))
        psum_pool = ctx.enter_context(tc.tile_pool(name="psum", bufs=2, space="PSUM"))
        copy_pool = ctx.enter_context(tc.tile_pool(name="copies", bufs=COPY_BUFS))
        vpool = ctx.enter_context(tc.tile_pool(name="vpool", bufs=1))
        outp = ctx.enter_context(tc.tile_pool(name="outp", bufs=1))

        saug = inputs.tile([K, NSH], bf16)
        taug = inputs.tile([K, M], bf16)
        # first-needed slices via the fast SP HWDGE path, rest on gpsimd
        nc.sync.dma_start(out=saug[:, :128], in_=saugT[:, :128])
        nc.sync.dma_start(out=taug[:, :512], in_=taugT[:, :512])
        nc.sync.dma_start(out=taug[:, 512:2048], in_=taugT[:, 512:2048])
        nc.gpsimd.dma_start(out=saug[:, 128:], in_=saugT[:, 128:])
        for i in range(2, 4):
            nc.gpsimd.dma_start(
                out=taug[:, i * (M // 4):(i + 1) * (M // 4)],
                in_=taugT[:, i * (M // 4):(i + 1) * (M // 4)],
            )

        V = [vpool.tile([128, TK, NSH], f16, name=f"V{h}") for h in range(HALVES)]
        junk = vpool.tile([128, 2048], f16)          # dead-write sink
        # touch the scalar engine immediately so its activation-table load
        # (1.3us) overlaps the input DMAs instead of delaying the 1st drain
        nc.vector.memset(junk[:, 0:8], 0.0)
        nc.scalar.copy(out=junk[:, 0:8], in_=junk[:, 0:8])

        rowpart = outp.tile([128, nrow_slots], f32)
        colpart = outp.tile([128, n_col_slots], f32)
        acc = vpool.tile([128, HALVES, CK * 128], f16)
        accB = vpool.tile([128, HALVES, CK * 128], f16)
        nc.gpsimd.memset(acc, CHAIN_INIT)

        pending = []          # (due_step, h, base_slot, k, r0, rl)
        col_slot = 0
        stage_idx = {0: 0, 1: 0}
        r_done = {0: 0, 1: 0}
        step = 0

        def flush_pending(now, force=False):
            nonlocal pending
            budget = len(pending) if force else SPREAD
            emitted = 0
            keep = []
            for (due, h, base, k, r0, rl) in pending:
                if (force or now >= due) and emitted < budget:
                    nc.vector.tensor_scalar(
                        out=junk[:, :rl],
                        in0=V[h][:, k, r0:r0 + rl],
                        scalar1=0.0,
                        scalar2=None,
                        op0=mybir.AluOpType.add,
                        op1=mybir.AluOpType.max,
                        accum_out=colpart[:, base + k:base + k + 1],
                    )
                    emitted += 1
                else:
                    keep.append((due, h, base, k, r0, rl))
            pending = keep

        deferred = []   # postponed DVE post-work: (rowmax_ap, slot, cph, h)

        def emit_deferred():
            for (dmax, dslot, dcph, dh) in deferred:
                nc.vector.tensor_scalar(
                    out=junk[:, :SPLIT], in0=dmax, scalar1=0.0,
                    scalar2=None, op0=mybir.AluOpType.add,
                    op1=mybir.AluOpType.max, accum_out=dslot,
                )
                a_sl = acc[:, dh, :]
                nc.vector.tensor_tensor(
                    out=a_sl, in0=dcph[:, TK * 128:], in1=a_sl,
                    op=mybir.AluOpType.max,
                )
            deferred.clear()

        tail_sorted = sorted(TAIL_TILES)
        for h in range(HALVES):
            for nt in range(NT):
                ps = psum_pool.tile([128, 2048], f32, tag="ps")
                for q in range(4):
                    mt = h * 4 + q
                    nc.tensor.matmul(
                        ps[:, q * 512:(q + 1) * 512],
                        saug[:, nt * 128:(nt + 1) * 128],
                        taug[:, mt * 512:(mt + 1) * 512],
                        start=True,
                        stop=True,
                    )
                cph = copy_pool.tile([128, 2048], f16, tag="cph")
                t_i = h * NT + nt
                slotA = rowpart[:, 2 * t_i:2 * t_i + 1]
                slotB = rowpart[:, 2 * t_i + 1:2 * t_i + 2]
                # DVE fused drain of the tail columns first (frees PSUM in
                # lockstep with the ACT drain of the head columns)
                nc.vector.tensor_scalar(
                    out=cph[:, SPLIT:], in0=ps[:, SPLIT:], scalar1=0.0,
                    scalar2=None, op0=mybir.AluOpType.add,
                    op1=mybir.AluOpType.max, accum_out=slotB,
                )
                nc.scalar.copy(out=cph[:, :SPLIT], in_=ps[:, :SPLIT])
                emit_deferred()
                deferred.append((cph[:, :SPLIT], slotA, cph, h))

                if (h, nt) in TAIL_TILES:
                    ti = tail_sorted.index((h, nt))
                    tred = copy_pool.tile([128, TK * 128], f16, tag="tred")
                    nc.gpsimd.partition_all_reduce(
                        tred, cph[:, :TK * 128], 128, bass_isa.ReduceOp.max
                    )
                    nc.sync.dma_start(
                        out=out_tail[0:1, ti * TK * 128:(ti + 1) * TK * 128],
                        in_=tred[0:1, :],
                    )
                else:
                    nc.sync.dma_start_transpose(
                        out=V[h][:, :, nt * 128:(nt + 1) * 128],
                        in_=cph[:, :TK * 128],
                    )

                step += 1
                if stage_idx[h] < len(STAGES[h]):
                    nt_end, r_len = STAGES[h][stage_idx[h]]
                    if nt + 1 == nt_end:
                        for k in range(TK):
                            pending.append((step + VRED_DELAY, h, col_slot,
                                            k, r_done[h], r_len))
                        col_slot += TK
                        r_done[h] += r_len
                        stage_idx[h] += 1
                flush_pending(step)

                if nt == NT - 1:
                    # this half's chain is complete: partition-reduce it now
                    emit_deferred()
                    nc.gpsimd.partition_all_reduce(
                        accB[:, h, :], acc[:, h, :], 128, bass_isa.ReduceOp.max
                    )
                    nc.sync.dma_start(
                        out=out_chain[0:1, h * CK * 128:(h + 1) * CK * 128],
                        in_=accB[0:1, h, :],
                    )

        emit_deferred()
        flush_pending(step, force=True)
        nc.sync.dma_start(out=out_row[:, :], in_=rowpart)
        nc.sync.dma_start(out=out_col[:, :], in_=colpart)

    nc.finalize()
    return nc


def _augment(source, target):
    """Per-core augmented bf16 hi/lo operands (NEGATED-distance space)."""
    bf = ml_dtypes.bfloat16

    def split(x):
        hi = x.astype(bf)
        lo = (x - hi.astype(np.float32)).astype(bf)
        return hi, lo

    in_maps = []
    for c in range(N_CORES):
        b, hh = c // 2, c % 2
        s = np.asarray(source[b, hh * NSH:(hh + 1) * NSH], dtype=np.float32)
        t = np.asarray(target[b], dtype=np.float32)
        a = 2.0 * s
        ns = -(s * s).sum(axis=1, dtype=np.float32)
        ntg = (t * t).sum(axis=1, dtype=np.float32)
        ah, al = split(a)
        th, tl = split(t)
        nsh_, nsl = split(ns)
        nth, ntl = split(ntg)
        ones_s = np.ones(NSH, dtype=bf)
        ones_t = np.ones(M, dtype=bf)

        sa = np.empty((K, NSH), dtype=bf)
        ta = np.empty((K, M), dtype=bf)
        sa[0:3] = ah.T
        ta[0:3] = th.T
        sa[3:6] = ah.T
        ta[3:6] = tl.T
        sa[6:9] = al.T
        ta[6:9] = th.T
        sa[9:12] = al.T
        ta[9:12] = tl.T
        sa[12] = nsh_
        sa[13] = nsl
        ta[12] = ones_t
        ta[13] = ones_t
        sa[14] = -ones_s
        sa[15] = -ones_s
        ta[14] = nth
        ta[15] = ntl

        in_maps.append({"saugT": sa, "taugT": ta})
    return in_maps


# test harness hook: set _BENCH["trace"]=True to profile; results land in
# _BENCH["last"] (BassKernelResults with exec_time_ns).
_BENCH = {"trace": False, "last": None}


def _core_mins(res):
    """Decode one core's outputs -> (rowmax_neg [NSH], colmax_neg [M])."""
    # rows: two slots per tile (ACT part / DVE fused part)
    rp = res["out_row"]                       # (128, 2*32)
    tile_rowmax = np.maximum(rp[:, 0::2], rp[:, 1::2])   # (128, 32) t=h*NT+nt
    # row r of half h lives at tile h*NT + r//128, partition r%128
    rowmax = np.maximum(tile_rowmax[:, :NT], tile_rowmax[:, NT:])  # (128, NT)
    rowmax_neg = rowmax.T.reshape(-1)         # (NSH,) ordered by r

    # columns
    cp = res["out_col"].astype(np.float32)    # (128, n_col_slots)
    tl = res["out_tail"][0].astype(np.float32)
    ch = res["out_chain"][0].astype(np.float32)
    colmax = np.full((M,), -np.inf, np.float32)
    # V-stage slot bases follow build order: h=0 stages, then h=1
    slot_base = {}
    csl = 0
    for h in range(HALVES):
        for s_i in range(len(STAGES[h])):
            slot_base[(h, s_i)] = csl
            csl += TK
    for h in range(HALVES):
        part = np.full((128, TK), -np.inf, np.float32)
        for s_i in range(len(STAGES[h])):
            base = slot_base[(h, s_i)]
            part = np.maximum(part, cp[:, base:base + TK])
        # col = h*2048 + k*128 + p  ->  part[p, k]
        colmax[h * 2048:h * 2048 + TK * 128] = part.T.reshape(-1)
        # chain blocks k in [TK, 16)
        colmax[h * 2048 + TK * 128:(h + 1) * 2048] = ch[h * CK * 128:(h + 1) * CK * 128]
    # pool-routed tiles: out_tail[ti] covers cols [h*2048, h*2048+TK*128)
    for ti, (h, nt) in enumerate(sorted(TAIL_TILES)):
        seg = tl[ti * TK * 128:(ti + 1) * TK * 128]
        sl = slice(h * 2048, h * 2048 + TK * 128)
        colmax[sl] = np.maximum(colmax[sl], seg)
    return rowmax_neg, colmax


def kernel(source, target):
    global _PROGRAM
    from concourse.bass_utils import run_bass_kernel_spmd

    source = np.asarray(source, dtype=np.float32)
    target = np.asarray(target, dtype=np.float32)

    if _PROGRAM is None:
        _PROGRAM = _build_program()

    in_maps = _augment(source, target)
    bkr = run_bass_kernel_spmd(
        _PROGRAM, in_maps, list(range(N_CORES)), trace=_BENCH["trace"]
    )
    _BENCH["last"] = bkr
    res = bkr.results

    loss = np.float64(0.0)
    for b in range(B):
        r0_row, r0_col = _core_mins(res[2 * b])
        r1_row, r1_col = _core_mins(res[2 * b + 1])
        rowmin = -np.concatenate([r0_row, r1_row])        # (N,)
        colmin = -np.maximum(r0_col, r1_col)              # (M,)
        loss += rowmin.mean(dtype=np.float64) + colmin.mean(dtype=np.float64)
    return np.float32(loss / B)


# revision 13
# speedup vs baseline: 1.0328x; 1.0022x over previous
"""Bidirectional Chamfer distance on 8 Trainium2 NeuronCores (v3).

Problem: B=4 batches, N=M=4096 3-D points, f32.
  dist[b,n,m] = ||s[b,n]-t[b,m]||^2
  loss = mean_b( mean_n min_m dist + mean_m min_n dist )

Sharding: core c handles batch b=c//2, source-row half hh=c%2
(2048 source rows x 4096 target cols per core).  All device math runs in
NEGATED-distance space (PE emits -dist via bf16 hi/lo augmented matmuls,
fp32-exact), so every reduction is a max.

v3 engine layout, per (h, nt) tile of [128 src rows x 2048 tgt cols]:
  PE    4 matmuls -> ps (PSUM fp32)                      ~0.85us
  drain split per route (1668 transposed / 1692 pool): ACT copies the
        head columns -> cph fp16 (~1.6us cadence); DVE drains the rest fused
        with its rowmax accum (frees PSUM in lockstep with ACT)
  row   DVE tensor_scalar 4x accum over the ACT part (out -> junk buf)
  col   three routes, balancing DVE/SP-DMA/Pool:
        - transposed tiles (nt 0..3 per half): SP-issued xbar transpose
          of cph[:, :1792] into V[h][128, 14, r]; per-column-class
          colmins come from staged DVE tensor_scalar 4x accums over V
        - pool tiles (nt 4..15, both halves): gpsimd partition_all_reduce
          of cph[:, :1792] directly (128-row max per column), keeping the
          transpose+V-reduce tail off the critical path
        - chain blocks (k=14,15): DVE tensor_tensor fp16 chain into acc,
          finished per-half by one partition_all_reduce
Stage reduces are throttled (<=3 per tile) and deferred behind each
tile's fused drain so DVE's in-order stream never stalls PSUM rotation.
CoreSim cost model: ~60.7us per core (ACT ~52us cadence-bound; DVE ~48,
SP-DMA ~25, Pool ~41 busy).
"""

import numpy as np
import ml_dtypes

B, N, M = 4, 4096, 4096
N_CORES = 8
NSH = N // 2          # 2048 source rows per core
K = 16                # augmented contraction dim
NT = NSH // 128       # 16 stationary tiles per half
HALVES = 2

TK = 14               # k-blocks (128 cols each) routed via transpose/pool
CK = 16 - TK          # k-blocks routed via the fp16 chain
SPLIT_T = 1668        # drain split for transposed tiles (ACT below, DVE above)
SPLIT_P = 1692        # drain split for pool-routed tiles
SPREAD = 3            # max stage reduces injected per tile step
VRED_DELAY = 2        # tiles to defer a completed stage's reduces
COPY_BUFS = 8
CHAIN_INIT = -60000.0
# V-reduce stages per half: list of (nt_end, r_len)
STAGES = {0: [(2, 256), (4, 256)], 1: [(2, 256), (4, 256)]}
# pool-routed tiles: columns [0, TK*128) reduced by partition_all_reduce
TAIL_TILES = (tuple((0, x) for x in range(4, 16))
              + tuple((1, x) for x in range(4, 16)))

_PROGRAM = None


def _build_program():
    import concourse.mybir as mybir
    import concourse.tile as tile
    from concourse import bacc, bass_isa
    from contextlib import ExitStack

    nc = bacc.Bacc(name="chamfer3")
    f32 = mybir.dt.float32
    f16 = mybir.dt.float16
    bf16 = mybir.dt.bfloat16

    nrow_slots = HALVES * NT * 2
    n_col_slots = sum(len(v) for v in STAGES.values()) * TK

    saugT = nc.dram_tensor("saugT", [K, NSH], bf16, kind="ExternalInput")
    taugT = nc.dram_tensor("taugT", [K, M], bf16, kind="ExternalInput")
    out_row = nc.dram_tensor("out_row", [128, nrow_slots], f32, kind="ExternalOutput")
    out_col = nc.dram_tensor("out_col", [128, n_col_slots], f32, kind="ExternalOutput")
    out_chain = nc.dram_tensor("out_chain", [1, HALVES * CK * 128], f16,
                               kind="ExternalOutput")
    out_tail = nc.dram_tensor("out_tail", [1, len(TAIL_TILES) * TK * 128], f16,
                              kind="ExternalOutput")

    with tile.TileContext(nc) as tc, ExitStack() as ctx:
        inputs = ctx.enter_context(tc.tile_pool(name="inputs", bufs=1))
        psum_pool = ctx.enter_context(tc.tile_pool(name="psum", bufs=2, space="PSUM"))
        copy_pool = ctx.enter_context(tc.tile_pool(name="copies", bufs=COPY_BUFS))
        vpool = ctx.enter_context(tc.tile_pool(name="vpool", bufs=1))
        outp = ctx.enter_context(tc.tile_pool(name="outp", bufs=1))

        saug = inputs.tile([K, NSH], bf16)
        taug = inputs.tile([K, M], bf16)
        # first-needed slices via the fast SP HWDGE path, rest on gpsimd
        nc.sync.dma_start(out=saug[:, :128], in_=saugT[:, :128])
        nc.sync.dma_start(out=taug[:, :512], in_=taugT[:, :512])
        nc.sync.dma_start(out=taug[:, 512:2048], in_=taugT[:, 512:2048])
        nc.gpsimd.dma_start(out=saug[:, 128:], in_=saugT[:, 128:])
        for i in range(2, 4):
            nc.gpsimd.dma_start(
                out=taug[:, i * (M // 4):(i + 1) * (M // 4)],
                in_=taugT[:, i * (M // 4):(i + 1) * (M // 4)],
            )

        V = [vpool.tile([128, TK, NSH], f16, name=f"V{h}") for h in range(HALVES)]
        junk = vpool.tile([128, 2048], f16)          # dead-write sink
        # touch the scalar engine immediately so its activation-table load
        # (1.3us) overlaps the input DMAs instead of delaying the 1st drain
        nc.vector.memset(junk[:, 0:8], 0.0)
        nc.scalar.copy(out=junk[:, 0:8], in_=junk[:, 0:8])

        rowpart = outp.tile([128, nrow_slots], f32)
        colpart = outp.tile([128, n_col_slots], f32)
        acc = vpool.tile([128, HALVES, CK * 128], f16)
        accB = vpool.tile([128, HALVES, CK * 128], f16)
        nc.gpsimd.memset(acc, CHAIN_INIT)

        pending = []          # (due_step, h, base_slot, k, r0, rl)
        col_slot = 0
        stage_idx = {0: 0, 1: 0}
        r_done = {0: 0, 1: 0}
        step = 0

        def flush_pending(now, force=False):
            nonlocal pending
            budget = len(pending) if force else SPREAD
            emitted = 0
            keep = []
            for (due, h, base, k, r0, rl) in pending:
                if (force or now >= due) and emitted < budget:
                    nc.vector.tensor_scalar(
                        out=junk[:, :rl],
                        in0=V[h][:, k, r0:r0 + rl],
                        scalar1=0.0,
                        scalar2=None,
                        op0=mybir.AluOpType.add,
                        op1=mybir.AluOpType.max,
                        accum_out=colpart[:, base + k:base + k + 1],
                    )
                    emitted += 1
                else:
                    keep.append((due, h, base, k, r0, rl))
            pending = keep

        deferred = []   # postponed DVE post-work: (rowmax_ap, slot, cph, h)

        def emit_deferred():
            for (dmax, dslot, dcph, dh) in deferred:
                nc.vector.tensor_scalar(
                    out=junk[:, :dmax.shape[-1]], in0=dmax, scalar1=0.0,
                    scalar2=None, op0=mybir.AluOpType.add,
                    op1=mybir.AluOpType.max, accum_out=dslot,
                )
                a_sl = acc[:, dh, :]
                nc.vector.tensor_tensor(
                    out=a_sl, in0=dcph[:, TK * 128:], in1=a_sl,
                    op=mybir.AluOpType.max,
                )
            deferred.clear()

        tail_sorted = sorted(TAIL_TILES)
        for h in range(HALVES):
            for nt in range(NT):
                ps = psum_pool.tile([128, 2048], f32, tag="ps")
                for q in range(4):
                    mt = h * 4 + q
                    nc.tensor.matmul(
                        ps[:, q * 512:(q + 1) * 512],
                        saug[:, nt * 128:(nt + 1) * 128],
                        taug[:, mt * 512:(mt + 1) * 512],
                        start=True,
                        stop=True,
                    )
                cph = copy_pool.tile([128, 2048], f16, tag="cph")
                t_i = h * NT + nt
                spl = SPLIT_P if (h, nt) in TAIL_TILES else SPLIT_T
                slotA = rowpart[:, 2 * t_i:2 * t_i + 1]
                slotB = rowpart[:, 2 * t_i + 1:2 * t_i + 2]
                # DVE fused drain of the tail columns first (frees PSUM in
                # lockstep with the ACT drain of the head columns)
                nc.vector.tensor_scalar(
                    out=cph[:, spl:], in0=ps[:, spl:], scalar1=0.0,
                    scalar2=None, op0=mybir.AluOpType.add,
                    op1=mybir.AluOpType.max, accum_out=slotB,
                )
                nc.scalar.copy(out=cph[:, :spl], in_=ps[:, :spl])
                emit_deferred()
                deferred.append((cph[:, :spl], slotA, cph, h))

                if (h, nt) in TAIL_TILES:
                    ti = tail_sorted.index((h, nt))
                    tred = copy_pool.tile([128, TK * 128], f16, tag="tred")
                    nc.gpsimd.partition_all_reduce(
                        tred, cph[:, :TK * 128], 128, bass_isa.ReduceOp.max
                    )
                    nc.sync.dma_start(
                        out=out_tail[0:1, ti * TK * 128:(ti + 1) * TK * 128],
                        in_=tred[0:1, :],
                    )
                else:
                    nc.sync.dma_start_transpose(
                        out=V[h][:, :, nt * 128:(nt + 1) * 128],
                        in_=cph[:, :TK * 128],
                    )

                step += 1
                if stage_idx[h] < len(STAGES[h]):
                    nt_end, r_len = STAGES[h][stage_idx[h]]
                    if nt + 1 == nt_end:
                        for k in range(TK):
                            pending.append((step + VRED_DELAY, h, col_slot,
                                            k, r_done[h], r_len))
                        col_slot += TK
                        r_done[h] += r_len
                        stage_idx[h] += 1
                flush_pending(step)

                if nt == NT - 1:
                    # this half's chain is complete: partition-reduce it now
                    emit_deferred()
                    nc.gpsimd.partition_all_reduce(
                        accB[:, h, :], acc[:, h, :], 128, bass_isa.ReduceOp.max
                    )
                    nc.sync.dma_start(
                        out=out_chain[0:1, h * CK * 128:(h + 1) * CK * 128],
                        in_=accB[0:1, h, :],
                    )

        emit_deferred()
        flush_pending(step, force=True)
        nc.sync.dma_start(out=out_row[:, :], in_=rowpart)
        nc.sync.dma_start(out=out_col[:, :], in_=colpart)

    nc.finalize()
    return nc


def _augment(source, target):
    """Per-core augmented bf16 hi/lo operands (NEGATED-distance space)."""
    bf = ml_dtypes.bfloat16

    def split(x):
        hi = x.astype(bf)
        lo = (x - hi.astype(np.float32)).astype(bf)
        return hi, lo

    in_maps = []
    for c in range(N_CORES):
        b, hh = c // 2, c % 2
        s = np.asarray(source[b, hh * NSH:(hh + 1) * NSH], dtype=np.float32)
        t = np.asarray(target[b], dtype=np.float32)
        a = 2.0 * s
        ns = -(s * s).sum(axis=1, dtype=np.float32)
        ntg = (t * t).sum(axis=1, dtype=np.float32)
        ah, al = split(a)
        th, tl = split(t)
        nsh_, nsl = split(ns)
        nth, ntl = split(ntg)
        ones_s = np.ones(NSH, dtype=bf)
        ones_t = np.ones(M, dtype=bf)

        sa = np.empty((K, NSH), dtype=bf)
        ta = np.empty((K, M), dtype=bf)
        sa[0:3] = ah.T
        ta[0:3] = th.T
        sa[3:6] = ah.T
        ta[3:6] = tl.T
        sa[6:9] = al.T
        ta[6:9] = th.T
        sa[9:12] = al.T
        ta[9:12] = tl.T
        sa[12] = nsh_
        sa[13] = nsl
        ta[12] = ones_t
        ta[13] = ones_t
        sa[14] = -ones_s
        sa[15] = -ones_s
        ta[14] = nth
        ta[15] = ntl

        in_maps.append({"saugT": sa, "taugT": ta})
    return in_maps


# test harness hook: set _BENCH["trace"]=True to profile; results land in
# _BENCH["last"] (BassKernelResults with exec_time_ns).
_BENCH = {"trace": False, "last": None}


def _core_mins(res):
    """Decode one core's outputs -> (rowmax_neg [NSH], colmax_neg [M])."""
    # rows: two slots per tile (ACT part / DVE fused part)
    rp = res["out_row"]                       # (128, 2*32)
    tile_rowmax = np.maximum(rp[:, 0::2], rp[:, 1::2])   # (128, 32) t=h*NT+nt
    # row r of half h lives at tile h*NT + r//128, partition r%128
    rowmax = np.maximum(tile_rowmax[:, :NT], tile_rowmax[:, NT:])  # (128, NT)
    rowmax_neg = rowmax.T.reshape(-1)         # (NSH,) ordered by r

    # columns
    cp = res["out_col"].astype(np.float32)    # (128, n_col_slots)
    tl = res["out_tail"][0].astype(np.float32)
    ch = res["out_chain"][0].astype(np.float32)
    colmax = np.full((M,), -np.inf, np.float32)
    # V-stage slot bases follow build order: h=0 stages, then h=1
    slot_base = {}
    csl = 0
    for h in range(HALVES):
        for s_i in range(len(STAGES[h])):
            slot_base[(h, s_i)] = csl
            csl += TK
    for h in range(HALVES):
        part = np.full((128, TK), -np.inf, np.float32)
        for s_i in range(len(STAGES[h])):
            base = slot_base[(h, s_i)]
            part = np.maximum(part, cp[:, base:base + TK])
        # col = h*2048 + k*128 + p  ->  part[p, k]
        colmax[h * 2048:h * 2048 + TK * 128] = part.T.reshape(-1)
        # chain blocks k in [TK, 16)
        colmax[h * 2048 + TK * 128:(h + 1) * 2048] = ch[h * CK * 128:(h + 1) * CK * 128]
    # pool-routed tiles: out_tail[ti] covers cols [h*2048, h*2048+TK*128)
    for ti, (h, nt) in enumerate(sorted(TAIL_TILES)):
        seg = tl[ti * TK * 128:(ti + 1) * TK * 128]
        sl = slice(h * 2048, h * 2048 + TK * 128)
        colmax[sl] = np.maximum(colmax[sl], seg)
    return rowmax_neg, colmax


def kernel(source, target):
    global _PROGRAM
    from concourse.bass_utils import run_bass_kernel_spmd

    source = np.asarray(source, dtype=np.float32)
    target = np.asarray(target, dtype=np.float32)

    if _PROGRAM is None:
        _PROGRAM = _build_program()

    in_maps = _augment(source, target)
    bkr = run_bass_kernel_spmd(
        _PROGRAM, in_maps, list(range(N_CORES)), trace=_BENCH["trace"]
    )
    _BENCH["last"] = bkr
    res = bkr.results

    loss = np.float64(0.0)
    for b in range(B):
        r0_row, r0_col = _core_mins(res[2 * b])
        r1_row, r1_col = _core_mins(res[2 * b + 1])
        rowmin = -np.concatenate([r0_row, r1_row])        # (N,)
        colmin = -np.maximum(r0_col, r1_col)              # (M,)
        loss += rowmin.mean(dtype=np.float64) + colmin.mean(dtype=np.float64)
    return np.float32(loss / B)


# revision 14
# speedup vs baseline: 1.0336x; 1.0008x over previous
"""Bidirectional Chamfer distance on 8 Trainium2 NeuronCores (v3).

Problem: B=4 batches, N=M=4096 3-D points, f32.
  dist[b,n,m] = ||s[b,n]-t[b,m]||^2
  loss = mean_b( mean_n min_m dist + mean_m min_n dist )

Sharding: core c handles batch b=c//2, source-row half hh=c%2
(2048 source rows x 4096 target cols per core).  All device math runs in
NEGATED-distance space (PE emits -dist via bf16 hi/lo augmented matmuls,
fp32-exact), so every reduction is a max.

v3 engine layout, per (h, nt) tile of [128 src rows x 2048 tgt cols]:
  PE    4 matmuls -> ps (PSUM fp32)                      ~0.85us
  drain split per route (1668 transposed / 1690 pool): ACT copies the
        head columns -> cph fp16 (~1.6us cadence); DVE drains the rest fused
        with its rowmax accum (frees PSUM in lockstep with ACT)
  row   DVE tensor_scalar 4x accum over the ACT part (out -> junk buf)
  col   three routes, balancing DVE/SP-DMA/Pool:
        - transposed tiles (nt 0..3 per half): SP-issued xbar transpose
          of cph[:, :1792] into V[h][128, 14, r]; per-column-class
          colmins come from staged DVE tensor_scalar 4x accums over V
        - pool tiles (nt 4..15, both halves): gpsimd partition_all_reduce
          of cph[:, :1792] directly (128-row max per column), keeping the
          transpose+V-reduce tail off the critical path
        - chain blocks (k=14,15): DVE tensor_tensor fp16 chain into acc,
          finished per-half by one partition_all_reduce
Stage reduces are throttled (<=3 per tile) and deferred behind each
tile's fused drain so DVE's in-order stream never stalls PSUM rotation.
CoreSim cost model: ~60.7us per core (ACT ~52us cadence-bound; DVE ~48,
SP-DMA ~25, Pool ~41 busy).
"""

import numpy as np
import ml_dtypes

B, N, M = 4, 4096, 4096
N_CORES = 8
NSH = N // 2          # 2048 source rows per core
K = 16                # augmented contraction dim
NT = NSH // 128       # 16 stationary tiles per half
HALVES = 2

TK = 14               # k-blocks (128 cols each) routed via transpose/pool
CK = 16 - TK          # k-blocks routed via the fp16 chain
SPLIT_T = 1668        # drain split for transposed tiles (ACT below, DVE above)
SPLIT_P = 1690        # drain split for pool-routed tiles
SPREAD = 3            # max stage reduces injected per tile step
VRED_DELAY = 2        # tiles to defer a completed stage's reduces
COPY_BUFS = 8
CHAIN_INIT = -60000.0
# V-reduce stages per half: list of (nt_end, r_len)
STAGES = {0: [(2, 256), (4, 256)], 1: [(2, 256), (4, 256)]}
# pool-routed tiles: columns [0, TK*128) reduced by partition_all_reduce
TAIL_TILES = (tuple((0, x) for x in range(4, 16))
              + tuple((1, x) for x in range(4, 16)))

_PROGRAM = None


def _build_program():
    import concourse.mybir as mybir
    import concourse.tile as tile
    from concourse import bacc, bass_isa
    from contextlib import ExitStack

    nc = bacc.Bacc(name="chamfer3")
    f32 = mybir.dt.float32
    f16 = mybir.dt.float16
    bf16 = mybir.dt.bfloat16

    nrow_slots = HALVES * NT * 2
    n_col_slots = sum(len(v) for v in STAGES.values()) * TK

    saugT = nc.dram_tensor("saugT", [K, NSH], bf16, kind="ExternalInput")
    taugT = nc.dram_tensor("taugT", [K, M], bf16, kind="ExternalInput")
    out_row = nc.dram_tensor("out_row", [128, nrow_slots], f32, kind="ExternalOutput")
    out_col = nc.dram_tensor("out_col", [128, n_col_slots], f32, kind="ExternalOutput")
    out_chain = nc.dram_tensor("out_chain", [1, HALVES * CK * 128], f16,
                               kind="ExternalOutput")
    out_tail = nc.dram_tensor("out_tail", [1, len(TAIL_TILES) * TK * 128], f16,
                              kind="ExternalOutput")

    with tile.TileContext(nc) as tc, ExitStack() as ctx:
        inputs = ctx.enter_context(tc.tile_pool(name="inputs", bufs=1))
        psum_pool = ctx.enter_context(tc.tile_pool(name="psum", bufs=2, space="PSUM"))
        copy_pool = ctx.enter_context(tc.tile_pool(name="copies", bufs=COPY_BUFS))
        vpool = ctx.enter_context(tc.tile_pool(name="vpool", bufs=1))
        outp = ctx.enter_context(tc.tile_pool(name="outp", bufs=1))

        saug = inputs.tile([K, NSH], bf16)
        taug = inputs.tile([K, M], bf16)
        # first-needed slices via the fast SP HWDGE path, rest on gpsimd
        nc.sync.dma_start(out=saug[:, :128], in_=saugT[:, :128])
        nc.sync.dma_start(out=taug[:, :512], in_=taugT[:, :512])
        nc.sync.dma_start(out=taug[:, 512:2048], in_=taugT[:, 512:2048])
        nc.gpsimd.dma_start(out=saug[:, 128:], in_=saugT[:, 128:])
        for i in range(2, 4):
            nc.gpsimd.dma_start(
                out=taug[:, i * (M // 4):(i + 1) * (M // 4)],
                in_=taugT[:, i * (M // 4):(i + 1) * (M // 4)],
            )

        V = [vpool.tile([128, TK, NSH], f16, name=f"V{h}") for h in range(HALVES)]
        junk = vpool.tile([128, 2048], f16)          # dead-write sink
        # touch the scalar engine immediately so its activation-table load
        # (1.3us) overlaps the input DMAs instead of delaying the 1st drain
        nc.vector.memset(junk[:, 0:8], 0.0)
        nc.scalar.copy(out=junk[:, 0:8], in_=junk[:, 0:8])

        rowpart = outp.tile([128, nrow_slots], f32)
        colpart = outp.tile([128, n_col_slots], f32)
        acc = vpool.tile([128, HALVES, CK * 128], f16)
        accB = vpool.tile([128, HALVES, CK * 128], f16)
        nc.gpsimd.memset(acc, CHAIN_INIT)

        pending = []          # (due_step, h, base_slot, k, r0, rl)
        col_slot = 0
        stage_idx = {0: 0, 1: 0}
        r_done = {0: 0, 1: 0}
        step = 0

        def flush_pending(now, force=False):
            nonlocal pending
            budget = len(pending) if force else SPREAD
            emitted = 0
            keep = []
            for (due, h, base, k, r0, rl) in pending:
                if (force or now >= due) and emitted < budget:
                    nc.vector.tensor_scalar(
                        out=junk[:, :rl],
                        in0=V[h][:, k, r0:r0 + rl],
                        scalar1=0.0,
                        scalar2=None,
                        op0=mybir.AluOpType.add,
                        op1=mybir.AluOpType.max,
                        accum_out=colpart[:, base + k:base + k + 1],
                    )
                    emitted += 1
                else:
                    keep.append((due, h, base, k, r0, rl))
            pending = keep

        deferred = []   # postponed DVE post-work: (rowmax_ap, slot, cph, h)

        def emit_deferred():
            for (dmax, dslot, dcph, dh) in deferred:
                nc.vector.tensor_scalar(
                    out=junk[:, :dmax.shape[-1]], in0=dmax, scalar1=0.0,
                    scalar2=None, op0=mybir.AluOpType.add,
                    op1=mybir.AluOpType.max, accum_out=dslot,
                )
                a_sl = acc[:, dh, :]
                nc.vector.tensor_tensor(
                    out=a_sl, in0=dcph[:, TK * 128:], in1=a_sl,
                    op=mybir.AluOpType.max,
                )
            deferred.clear()

        tail_sorted = sorted(TAIL_TILES)
        for h in range(HALVES):
            for nt in range(NT):
                ps = psum_pool.tile([128, 2048], f32, tag="ps")
                for q in range(4):
                    mt = h * 4 + q
                    nc.tensor.matmul(
                        ps[:, q * 512:(q + 1) * 512],
                        saug[:, nt * 128:(nt + 1) * 128],
                        taug[:, mt * 512:(mt + 1) * 512],
                        start=True,
                        stop=True,
                    )
                cph = copy_pool.tile([128, 2048], f16, tag="cph")
                t_i = h * NT + nt
                spl = SPLIT_P if (h, nt) in TAIL_TILES else SPLIT_T
                slotA = rowpart[:, 2 * t_i:2 * t_i + 1]
                slotB = rowpart[:, 2 * t_i + 1:2 * t_i + 2]
                # DVE fused drain of the tail columns first (frees PSUM in
                # lockstep with the ACT drain of the head columns)
                nc.vector.tensor_scalar(
                    out=cph[:, spl:], in0=ps[:, spl:], scalar1=0.0,
                    scalar2=None, op0=mybir.AluOpType.add,
                    op1=mybir.AluOpType.max, accum_out=slotB,
                )
                nc.scalar.copy(out=cph[:, :spl], in_=ps[:, :spl])
                emit_deferred()
                deferred.append((cph[:, :spl], slotA, cph, h))

                if (h, nt) in TAIL_TILES:
                    ti = tail_sorted.index((h, nt))
                    tred = copy_pool.tile([128, TK * 128], f16, tag="tred")
                    nc.gpsimd.partition_all_reduce(
                        tred, cph[:, :TK * 128], 128, bass_isa.ReduceOp.max
                    )
                    nc.sync.dma_start(
                        out=out_tail[0:1, ti * TK * 128:(ti + 1) * TK * 128],
                        in_=tred[0:1, :],
                    )
                else:
                    nc.sync.dma_start_transpose(
                        out=V[h][:, :, nt * 128:(nt + 1) * 128],
                        in_=cph[:, :TK * 128],
                    )

                step += 1
                if stage_idx[h] < len(STAGES[h]):
                    nt_end, r_len = STAGES[h][stage_idx[h]]
                    if nt + 1 == nt_end:
                        for k in range(TK):
                            pending.append((step + VRED_DELAY, h, col_slot,
                                            k, r_done[h], r_len))
                        col_slot += TK
                        r_done[h] += r_len
                        stage_idx[h] += 1
                flush_pending(step)

                if nt == NT - 1:
                    # this half's chain is complete: partition-reduce it now
                    emit_deferred()
                    nc.gpsimd.partition_all_reduce(
                        accB[:, h, :], acc[:, h, :], 128, bass_isa.ReduceOp.max
                    )
                    nc.sync.dma_start(
                        out=out_chain[0:1, h * CK * 128:(h + 1) * CK * 128],
                        in_=accB[0:1, h, :],
                    )

        emit_deferred()
        flush_pending(step, force=True)
        nc.sync.dma_start(out=out_row[:, :], in_=rowpart)
        nc.sync.dma_start(out=out_col[:, :], in_=colpart)

    nc.finalize()
    return nc


def _augment(source, target):
    """Per-core augmented bf16 hi/lo operands (NEGATED-distance space)."""
    bf = ml_dtypes.bfloat16

    def split(x):
        hi = x.astype(bf)
        lo = (x - hi.astype(np.float32)).astype(bf)
        return hi, lo

    in_maps = []
    for c in range(N_CORES):
        b, hh = c // 2, c % 2
        s = np.asarray(source[b, hh * NSH:(hh + 1) * NSH], dtype=np.float32)
        t = np.asarray(target[b], dtype=np.float32)
        a = 2.0 * s
        ns = -(s * s).sum(axis=1, dtype=np.float32)
        ntg = (t * t).sum(axis=1, dtype=np.float32)
        ah, al = split(a)
        th, tl = split(t)
        nsh_, nsl = split(ns)
        nth, ntl = split(ntg)
        ones_s = np.ones(NSH, dtype=bf)
        ones_t = np.ones(M, dtype=bf)

        sa = np.empty((K, NSH), dtype=bf)
        ta = np.empty((K, M), dtype=bf)
        sa[0:3] = ah.T
        ta[0:3] = th.T
        sa[3:6] = ah.T
        ta[3:6] = tl.T
        sa[6:9] = al.T
        ta[6:9] = th.T
        sa[9:12] = al.T
        ta[9:12] = tl.T
        sa[12] = nsh_
        sa[13] = nsl
        ta[12] = ones_t
        ta[13] = ones_t
        sa[14] = -ones_s
        sa[15] = -ones_s
        ta[14] = nth
        ta[15] = ntl

        in_maps.append({"saugT": sa, "taugT": ta})
    return in_maps


# test harness hook: set _BENCH["trace"]=True to profile; results land in
# _BENCH["last"] (BassKernelResults with exec_time_ns).
_BENCH = {"trace": False, "last": None}


def _core_mins(res):
    """Decode one core's outputs -> (rowmax_neg [NSH], colmax_neg [M])."""
    # rows: two slots per tile (ACT part / DVE fused part)
    rp = res["out_row"]                       # (128, 2*32)
    tile_rowmax = np.maximum(rp[:, 0::2], rp[:, 1::2])   # (128, 32) t=h*NT+nt
    # row r of half h lives at tile h*NT + r//128, partition r%128
    rowmax = np.maximum(tile_rowmax[:, :NT], tile_rowmax[:, NT:])  # (128, NT)
    rowmax_neg = rowmax.T.reshape(-1)         # (NSH,) ordered by r

    # columns
    cp = res["out_col"].astype(np.float32)    # (128, n_col_slots)
    tl = res["out_tail"][0].astype(np.float32)
    ch = res["out_chain"][0].astype(np.float32)
    colmax = np.full((M,), -np.inf, np.float32)
    # V-stage slot bases follow build order: h=0 stages, then h=1
    slot_base = {}
    csl = 0
    for h in range(HALVES):
        for s_i in range(len(STAGES[h])):
            slot_base[(h, s_i)] = csl
            csl += TK
    for h in range(HALVES):
        part = np.full((128, TK), -np.inf, np.float32)
        for s_i in range(len(STAGES[h])):
            base = slot_base[(h, s_i)]
            part = np.maximum(part, cp[:, base:base + TK])
        # col = h*2048 + k*128 + p  ->  part[p, k]
        colmax[h * 2048:h * 2048 + TK * 128] = part.T.reshape(-1)
        # chain blocks k in [TK, 16)
        colmax[h * 2048 + TK * 128:(h + 1) * 2048] = ch[h * CK * 128:(h + 1) * CK * 128]
    # pool-routed tiles: out_tail[ti] covers cols [h*2048, h*2048+TK*128)
    for ti, (h, nt) in enumerate(sorted(TAIL_TILES)):
        seg = tl[ti * TK * 128:(ti + 1) * TK * 128]
        sl = slice(h * 2048, h * 2048 + TK * 128)
        colmax[sl] = np.maximum(colmax[sl], seg)
    return rowmax_neg, colmax


def kernel(source, target):
    global _PROGRAM
    from concourse.bass_utils import run_bass_kernel_spmd

    source = np.asarray(source, dtype=np.float32)
    target = np.asarray(target, dtype=np.float32)

    if _PROGRAM is None:
        _PROGRAM = _build_program()

    in_maps = _augment(source, target)
    bkr = run_bass_kernel_spmd(
        _PROGRAM, in_maps, list(range(N_CORES)), trace=_BENCH["trace"]
    )
    _BENCH["last"] = bkr
    res = bkr.results

    loss = np.float64(0.0)
    for b in range(B):
        r0_row, r0_col = _core_mins(res[2 * b])
        r1_row, r1_col = _core_mins(res[2 * b + 1])
        rowmin = -np.concatenate([r0_row, r1_row])        # (N,)
        colmin = -np.maximum(r0_col, r1_col)              # (M,)
        loss += rowmin.mean(dtype=np.float64) + colmin.mean(dtype=np.float64)
    return np.float32(loss / B)


# revision 15
# speedup vs baseline: 1.0372x; 1.0036x over previous
"""Bidirectional Chamfer distance on 8 Trainium2 NeuronCores (v3).

Problem: B=4 batches, N=M=4096 3-D points, f32.
  dist[b,n,m] = ||s[b,n]-t[b,m]||^2
  loss = mean_b( mean_n min_m dist + mean_m min_n dist )

Sharding: core c handles batch b=c//2, source-row half hh=c%2
(2048 source rows x 4096 target cols per core).  All device math runs in
NEGATED-distance space (PE emits -dist via bf16 hi/lo augmented matmuls,
fp32-exact), so every reduction is a max.

v3 engine layout, per (h, nt) tile of [128 src rows x 2048 tgt cols]:
  PE    4 matmuls -> ps (PSUM fp32)                      ~0.85us
  drain split per route (1668 transposed / 1690 pool): ACT copies the
        head columns -> cph fp16 (~1.6us cadence); DVE drains the rest fused
        with its rowmax accum (frees PSUM in lockstep with ACT)
  row   DVE tensor_scalar 4x accum over the ACT part (out -> junk buf)
  col   three routes, balancing DVE/SP-DMA/Pool:
        - transposed tiles (nt 0..3 per half): SP-issued xbar transpose
          of cph[:, :1792] into V[h][128, 14, r]; per-column-class
          colmins come from staged DVE tensor_scalar 4x accums over V
        - pool tiles (nt 4..15, both halves): gpsimd partition_all_reduce
          of cph[:, :1792] directly (128-row max per column), keeping the
          transpose+V-reduce tail off the critical path
        - chain blocks (k=14,15): DVE tensor_tensor fp16 chain into acc,
          finished per-half by one partition_all_reduce
Stage reduces are throttled (<=3 per tile) and deferred behind each
tile's fused drain so DVE's in-order stream never stalls PSUM rotation.
CoreSim cost model: ~60.7us per core (ACT ~52us cadence-bound; DVE ~48,
SP-DMA ~25, Pool ~41 busy).
"""

import numpy as np
import ml_dtypes

B, N, M = 4, 4096, 4096
N_CORES = 8
NSH = N // 2          # 2048 source rows per core
K = 16                # augmented contraction dim
NT = NSH // 128       # 16 stationary tiles per half
HALVES = 2

TK = 14               # k-blocks (128 cols each) routed via transpose/pool
CK = 16 - TK          # k-blocks routed via the fp16 chain
SPLIT_T = 1668        # drain split for transposed tiles (ACT below, DVE above)
SPLIT_P = 1690        # drain split for pool-routed tiles
SPREAD = 3            # max stage reduces injected per tile step
VRED_DELAY = 2        # tiles to defer a completed stage's reduces
COPY_BUFS = 8
CHAIN_INIT = -60000.0
# V-reduce stages per half: list of (nt_end, r_len)
STAGES = {0: [(2, 256), (4, 256)], 1: [(2, 256), (4, 256)]}
# pool-routed tiles: columns [0, TK*128) reduced by partition_all_reduce
TAIL_TILES = (tuple((0, x) for x in range(4, 16))
              + tuple((1, x) for x in range(4, 16)))

_PROGRAM = None


def _build_program():
    import concourse.mybir as mybir
    import concourse.tile as tile
    from concourse import bacc, bass_isa
    from contextlib import ExitStack

    nc = bacc.Bacc(name="chamfer3")
    f32 = mybir.dt.float32
    f16 = mybir.dt.float16
    bf16 = mybir.dt.bfloat16

    nrow_slots = HALVES * NT * 2
    n_col_slots = sum(len(v) for v in STAGES.values()) * TK

    saugT = nc.dram_tensor("saugT", [K, NSH], bf16, kind="ExternalInput")
    taugT = nc.dram_tensor("taugT", [K, M], bf16, kind="ExternalInput")
    out_row = nc.dram_tensor("out_row", [128, nrow_slots], f32, kind="ExternalOutput")
    out_col = nc.dram_tensor("out_col", [128, n_col_slots], f32, kind="ExternalOutput")
    out_chain = nc.dram_tensor("out_chain", [1, HALVES * CK * 128], f16,
                               kind="ExternalOutput")
    out_tail = nc.dram_tensor("out_tail", [1, len(TAIL_TILES) * TK * 128], f16,
                              kind="ExternalOutput")

    with tile.TileContext(nc) as tc, ExitStack() as ctx:
        inputs = ctx.enter_context(tc.tile_pool(name="inputs", bufs=1))
        psum_pool = ctx.enter_context(tc.tile_pool(name="psum", bufs=2, space="PSUM"))
        copy_pool = ctx.enter_context(tc.tile_pool(name="copies", bufs=COPY_BUFS))
        vpool = ctx.enter_context(tc.tile_pool(name="vpool", bufs=1))
        outp = ctx.enter_context(tc.tile_pool(name="outp", bufs=1))

        saug = inputs.tile([K, NSH], bf16)
        taug = inputs.tile([K, M], bf16)
        # first-needed slices via the fast SP HWDGE path, rest on gpsimd
        nc.sync.dma_start(out=saug[:, :128], in_=saugT[:, :128])
        nc.sync.dma_start(out=taug[:, :512], in_=taugT[:, :512])
        nc.sync.dma_start(out=taug[:, 512:2048], in_=taugT[:, 512:2048])
        nc.gpsimd.dma_start(out=saug[:, 128:], in_=saugT[:, 128:])
        for i in range(2, 4):
            nc.gpsimd.dma_start(
                out=taug[:, i * (M // 4):(i + 1) * (M // 4)],
                in_=taugT[:, i * (M // 4):(i + 1) * (M // 4)],
            )

        V = [vpool.tile([128, TK, NSH], f16, name=f"V{h}") for h in range(HALVES)]
        junk = vpool.tile([128, 2048], f16)          # dead-write sink
        # touch the scalar engine immediately so its activation-table load
        # (1.3us) overlaps the input DMAs instead of delaying the 1st drain
        nc.vector.memset(junk[:, 0:8], 0.0)
        nc.scalar.copy(out=junk[:, 0:8], in_=junk[:, 0:8])

        rowpart = outp.tile([128, nrow_slots], f32)
        colpart = outp.tile([128, n_col_slots], f32)
        acc = vpool.tile([128, HALVES, CK * 128], f16)
        accB = vpool.tile([128, HALVES, CK * 128], f16)
        nc.gpsimd.memset(acc, CHAIN_INIT)

        pending = []          # (due_step, h, base_slot, k, r0, rl)
        col_slot = 0
        stage_idx = {0: 0, 1: 0}
        r_done = {0: 0, 1: 0}
        step = 0

        def flush_pending(now, force=False):
            nonlocal pending
            budget = len(pending) if force else SPREAD
            emitted = 0
            keep = []
            for (due, h, base, k, r0, rl) in pending:
                if (force or now >= due) and emitted < budget:
                    nc.vector.tensor_scalar(
                        out=junk[:, :rl],
                        in0=V[h][:, k, r0:r0 + rl],
                        scalar1=0.0,
                        scalar2=None,
                        op0=mybir.AluOpType.add,
                        op1=mybir.AluOpType.max,
                        accum_out=colpart[:, base + k:base + k + 1],
                    )
                    emitted += 1
                else:
                    keep.append((due, h, base, k, r0, rl))
            pending = keep

        deferred = []   # postponed DVE post-work: (rowmax_ap, slot, cph, h)

        def emit_deferred():
            for (dmax, dslot, dcph, dh) in deferred:
                nc.vector.tensor_scalar(
                    out=junk[:, :dmax.shape[-1]], in0=dmax, scalar1=0.0,
                    scalar2=None, op0=mybir.AluOpType.add,
                    op1=mybir.AluOpType.max, accum_out=dslot,
                )
                a_sl = acc[:, dh, :]
                nc.vector.tensor_tensor(
                    out=a_sl, in0=dcph[:, TK * 128:], in1=a_sl,
                    op=mybir.AluOpType.max,
                )
            deferred.clear()

        tail_sorted = sorted(TAIL_TILES)
        for h in range(HALVES):
            for nt in range(NT):
                ps = psum_pool.tile([128, 2048], f32, tag="ps")
                # chunk 3 first: the DVE fused drain reads [SPLIT:, ] which
                # lies in chunk 3, so it can start ~0.6us before chunks 0-2
                # finish; the ACT drain waits those regardless
                for q in (3, 0, 1, 2):
                    mt = h * 4 + q
                    nc.tensor.matmul(
                        ps[:, q * 512:(q + 1) * 512],
                        saug[:, nt * 128:(nt + 1) * 128],
                        taug[:, mt * 512:(mt + 1) * 512],
                        start=True,
                        stop=True,
                    )
                cph = copy_pool.tile([128, 2048], f16, tag="cph")
                t_i = h * NT + nt
                spl = SPLIT_P if (h, nt) in TAIL_TILES else SPLIT_T
                slotA = rowpart[:, 2 * t_i:2 * t_i + 1]
                slotB = rowpart[:, 2 * t_i + 1:2 * t_i + 2]
                # DVE fused drain of the tail columns first (frees PSUM in
                # lockstep with the ACT drain of the head columns)
                nc.vector.tensor_scalar(
                    out=cph[:, spl:], in0=ps[:, spl:], scalar1=0.0,
                    scalar2=None, op0=mybir.AluOpType.add,
                    op1=mybir.AluOpType.max, accum_out=slotB,
                )
                nc.scalar.copy(out=cph[:, :spl], in_=ps[:, :spl])
                emit_deferred()
                deferred.append((cph[:, :spl], slotA, cph, h))

                if (h, nt) in TAIL_TILES:
                    ti = tail_sorted.index((h, nt))
                    tred = copy_pool.tile([128, TK * 128], f16, tag="tred")
                    nc.gpsimd.partition_all_reduce(
                        tred, cph[:, :TK * 128], 128, bass_isa.ReduceOp.max
                    )
                    nc.sync.dma_start(
                        out=out_tail[0:1, ti * TK * 128:(ti + 1) * TK * 128],
                        in_=tred[0:1, :],
                    )
                else:
                    nc.sync.dma_start_transpose(
                        out=V[h][:, :, nt * 128:(nt + 1) * 128],
                        in_=cph[:, :TK * 128],
                    )

                step += 1
                if stage_idx[h] < len(STAGES[h]):
                    nt_end, r_len = STAGES[h][stage_idx[h]]
                    if nt + 1 == nt_end:
                        for k in range(TK):
                            pending.append((step + VRED_DELAY, h, col_slot,
                                            k, r_done[h], r_len))
                        col_slot += TK
                        r_done[h] += r_len
                        stage_idx[h] += 1
                flush_pending(step)

                if nt == NT - 1:
                    # this half's chain is complete: partition-reduce it now
                    emit_deferred()
                    nc.gpsimd.partition_all_reduce(
                        accB[:, h, :], acc[:, h, :], 128, bass_isa.ReduceOp.max
                    )
                    nc.sync.dma_start(
                        out=out_chain[0:1, h * CK * 128:(h + 1) * CK * 128],
                        in_=accB[0:1, h, :],
                    )

        emit_deferred()
        flush_pending(step, force=True)
        nc.sync.dma_start(out=out_row[:, :], in_=rowpart)
        nc.sync.dma_start(out=out_col[:, :], in_=colpart)

    nc.finalize()
    return nc


def _augment(source, target):
    """Per-core augmented bf16 hi/lo operands (NEGATED-distance space)."""
    bf = ml_dtypes.bfloat16

    def split(x):
        hi = x.astype(bf)
        lo = (x - hi.astype(np.float32)).astype(bf)
        return hi, lo

    in_maps = []
    for c in range(N_CORES):
        b, hh = c // 2, c % 2
        s = np.asarray(source[b, hh * NSH:(hh + 1) * NSH], dtype=np.float32)
        t = np.asarray(target[b], dtype=np.float32)
        a = 2.0 * s
        ns = -(s * s).sum(axis=1, dtype=np.float32)
        ntg = (t * t).sum(axis=1, dtype=np.float32)
        ah, al = split(a)
        th, tl = split(t)
        nsh_, nsl = split(ns)
        nth, ntl = split(ntg)
        ones_s = np.ones(NSH, dtype=bf)
        ones_t = np.ones(M, dtype=bf)

        sa = np.empty((K, NSH), dtype=bf)
        ta = np.empty((K, M), dtype=bf)
        sa[0:3] = ah.T
        ta[0:3] = th.T
        sa[3:6] = ah.T
        ta[3:6] = tl.T
        sa[6:9] = al.T
        ta[6:9] = th.T
        sa[9:12] = al.T
        ta[9:12] = tl.T
        sa[12] = nsh_
        sa[13] = nsl
        ta[12] = ones_t
        ta[13] = ones_t
        sa[14] = -ones_s
        sa[15] = -ones_s
        ta[14] = nth
        ta[15] = ntl

        in_maps.append({"saugT": sa, "taugT": ta})
    return in_maps


# test harness hook: set _BENCH["trace"]=True to profile; results land in
# _BENCH["last"] (BassKernelResults with exec_time_ns).
_BENCH = {"trace": False, "last": None}


def _core_mins(res):
    """Decode one core's outputs -> (rowmax_neg [NSH], colmax_neg [M])."""
    # rows: two slots per tile (ACT part / DVE fused part)
    rp = res["out_row"]                       # (128, 2*32)
    tile_rowmax = np.maximum(rp[:, 0::2], rp[:, 1::2])   # (128, 32) t=h*NT+nt
    # row r of half h lives at tile h*NT + r//128, partition r%128
    rowmax = np.maximum(tile_rowmax[:, :NT], tile_rowmax[:, NT:])  # (128, NT)
    rowmax_neg = rowmax.T.reshape(-1)         # (NSH,) ordered by r

    # columns
    cp = res["out_col"].astype(np.float32)    # (128, n_col_slots)
    tl = res["out_tail"][0].astype(np.float32)
    ch = res["out_chain"][0].astype(np.float32)
    colmax = np.full((M,), -np.inf, np.float32)
    # V-stage slot bases follow build order: h=0 stages, then h=1
    slot_base = {}
    csl = 0
    for h in range(HALVES):
        for s_i in range(len(STAGES[h])):
            slot_base[(h, s_i)] = csl
            csl += TK
    for h in range(HALVES):
        part = np.full((128, TK), -np.inf, np.float32)
        for s_i in range(len(STAGES[h])):
            base = slot_base[(h, s_i)]
            part = np.maximum(part, cp[:, base:base + TK])
        # col = h*2048 + k*128 + p  ->  part[p, k]
        colmax[h * 2048:h * 2048 + TK * 128] = part.T.reshape(-1)
        # chain blocks k in [TK, 16)
        colmax[h * 2048 + TK * 128:(h + 1) * 2048] = ch[h * CK * 128:(h + 1) * CK * 128]
    # pool-routed tiles: out_tail[ti] covers cols [h*2048, h*2048+TK*128)
    for ti, (h, nt) in enumerate(sorted(TAIL_TILES)):
        seg = tl[ti * TK * 128:(ti + 1) * TK * 128]
        sl = slice(h * 2048, h * 2048 + TK * 128)
        colmax[sl] = np.maximum(colmax[sl], seg)
    return rowmax_neg, colmax


def kernel(source, target):
    global _PROGRAM
    from concourse.bass_utils import run_bass_kernel_spmd

    source = np.asarray(source, dtype=np.float32)
    target = np.asarray(target, dtype=np.float32)

    if _PROGRAM is None:
        _PROGRAM = _build_program()

    in_maps = _augment(source, target)
    bkr = run_bass_kernel_spmd(
        _PROGRAM, in_maps, list(range(N_CORES)), trace=_BENCH["trace"]
    )
    _BENCH["last"] = bkr
    res = bkr.results

    loss = np.float64(0.0)
    for b in range(B):
        r0_row, r0_col = _core_mins(res[2 * b])
        r1_row, r1_col = _core_mins(res[2 * b + 1])
        rowmin = -np.concatenate([r0_row, r1_row])        # (N,)
        colmin = -np.maximum(r0_col, r1_col)              # (M,)
        loss += rowmin.mean(dtype=np.float64) + colmin.mean(dtype=np.float64)
    return np.float32(loss / B)
